# revision 1
# baseline (speedup 1.0000x reference)
"""Distributed GATv1 (2x GAT + SAGE + MLP head) for Trainium2, 8 NeuronCores.

Strategy (graph/data parallel, per sharding hint):
- Nodes are sharded contiguously across the 8 cores; each core's local nodes
  are re-binned into tiles of 128 ("dst bins") balanced by in-degree so every
  bin has nearly the same number of incoming edges.
- Per GAT layer: a sharded dense phase computes g = [h | al_src] and al_dst
  for local nodes, then an AllGather replicates g; the edge phase gathers
  g[src] rows with indirect DMA, computes softmax weights
  w = exp(leaky_relu(al_s + al_d)) per edge (numerically safe without the
  max-subtraction since |logits| are O(1)), scales the gathered rows, and
  aggregates messages per dst bin with a one-hot "routing" matmul that also
  accumulates the softmax denominators as 3 extra columns.
- SAGE mean-aggregation reuses the same machinery with unit weights; its
  linear layers and the whole MLP head collapse into two [192,16] matmuls
  (no nonlinearity in between), folded on the host.
"""

import numpy as np

# Problem constants (hardcoded; kernel.py must be self-contained).
N = 50000
E = 800000
IN_C = 128
HID = 64
HEADS = 3
OUT_C = 16
C = HEADS * HID          # 192
ROW = C + HEADS          # 195 = [h | al_s]
NCORES = 8
P = 128


def _ceil(a, b):
    return -(-a - 0) // b if False else -(-a // b)


def _pack_bins(deg, nbins):
    """Greedy balanced binning: assign n=nbins*128 nodes to bins of 128 slots,
    minimizing the max per-bin edge count. Returns (bin_of, slot_of)."""
    n = len(deg)
    assert n == nbins * P
    order = np.argsort(-deg, kind="stable")
    bin_load = np.zeros(nbins, np.int64)
    bin_fill = np.zeros(nbins, np.int64)
    bin_of = np.zeros(n, np.int32)
    slot_of = np.zeros(n, np.int32)
    big = np.int64(1 << 60)
    for l in order:
        cand = np.where(bin_fill < P, bin_load, big)
        b = int(np.argmin(cand))
        bin_of[l] = b
        slot_of[l] = bin_fill[b]
        bin_fill[b] += 1
        bin_load[b] += deg[l]
    assert (bin_fill == P).all()
    return bin_of, slot_of


def _bucket_edges(e_src_pg, e_dstperm, nbins, int_extra=None):
    """Bucket edges by dst bin into [nbins, P, T] arrays (T = max needed).
    Returns (T, src_a[nbins,P,T] i32, slot_a[nbins,P,T] f32, extra_a or None)."""
    ebin = e_dstperm // P
    eslot = (e_dstperm % P).astype(np.float32)
    counts = np.bincount(ebin, minlength=nbins)
    T = max(1, _ceil(int(counts.max()), P))
    order = np.argsort(ebin, kind="stable")
    starts = np.zeros(nbins + 1, np.int64)
    starts[1:] = np.cumsum(counts)
    src_a = np.zeros((nbins, P * T), np.int32)
    slot_a = np.full((nbins, P * T), -1.0, np.float32)
    extra_a = None if int_extra is None else np.zeros((nbins, P * T), np.int32)
    for t in range(nbins):
        sel = order[starts[t]:starts[t + 1]]
        cnt = len(sel)
        src_a[t, :cnt] = e_src_pg[sel]
        slot_a[t, :cnt] = eslot[sel]
        if int_extra is not None:
            extra_a[t, :cnt] = int_extra[sel]
    src_a = src_a.reshape(nbins, P, T)
    slot_a = slot_a.reshape(nbins, P, T)
    if extra_a is not None:
        extra_a = extra_a.reshape(nbins, P, T)
    return T, src_a, slot_a, extra_a


def preprocess(x, edge_index, n_nodes, n_cores):
    """Host-side index preprocessing. Returns (cfg dict, per-core data dict)."""
    src = np.asarray(edge_index[0], np.int64)
    dst = np.asarray(edge_index[1], np.int64)
    NPC = n_nodes // n_cores
    NPpad = _ceil(NPC, P) * P
    NT = NPpad // P

    x = np.asarray(x, np.float32)
    owner = dst // NPC

    # degrees for packing: in-degree + 1 (self loop)
    deg = np.bincount(dst, minlength=n_nodes).astype(np.int64) + 1

    ggid = np.zeros(n_nodes, np.int64)   # global -> padded-global permuted id
    pad_perm = []                        # per core: permuted local ids of pad slots
    for k in range(n_cores):
        lo, hi = k * NPC, (k + 1) * NPC
        degs = np.concatenate([deg[lo:hi], np.ones(NPpad - NPC, np.int64)])
        b, s = _pack_bins(degs, NT)
        ggid[lo:hi] = k * NPpad + b[:NPC].astype(np.int64) * P + s[:NPC]
        pad_perm.append(b[NPC:].astype(np.int64) * P + s[NPC:])

    cores = []
    T_gat_all, T_sage_all = 1, 1
    per_core_raw = []
    for k in range(n_cores):
        m = owner == k
        es, ed = src[m], dst[m]
        # GAT edges: + self loops for real locals, + 1 fake edge per pad slot
        sl_nodes = np.arange(k * NPC, (k + 1) * NPC, dtype=np.int64)
        ges = np.concatenate([es, sl_nodes])
        ged = np.concatenate([ed, sl_nodes])
        g_src_pg = ggid[ges]
        g_dstperm = ggid[ged] - k * NPpad
        if len(pad_perm[k]):
            g_src_pg = np.concatenate(
                [g_src_pg, np.full(len(pad_perm[k]), ggid[0], np.int64)])
            g_dstperm = np.concatenate([g_dstperm, pad_perm[k]])
        # SAGE edges: raw edges only
        s_src_pg = ggid[es]
        s_dstperm = ggid[ed] - k * NPpad
        per_core_raw.append((g_src_pg, g_dstperm, s_src_pg, s_dstperm))
        T_gat_all = max(T_gat_all, _ceil(int(np.bincount(
            g_dstperm // P, minlength=NT).max()), P))
        T_sage_all = max(T_sage_all, _ceil(max(1, int(np.bincount(
            s_dstperm // P, minlength=NT).max())), P))

    for k in range(n_cores):
        g_src_pg, g_dstperm, s_src_pg, s_dstperm = per_core_raw[k]
        Tg, gsrc_a, gslot_a, gdst_a = _bucket_edges(
            g_src_pg, g_dstperm, NT, int_extra=g_dstperm)
        Ts, ssrc_a, sslot_a, _ = _bucket_edges(s_src_pg, s_dstperm, NT)
        # pad to uniform T across cores
        if Tg < T_gat_all:
            pad = T_gat_all - Tg
            gsrc_a = np.concatenate([gsrc_a, np.zeros((NT, P, pad), np.int32)], 2)
            gslot_a = np.concatenate([gslot_a, np.full((NT, P, pad), -1.0, np.float32)], 2)
            gdst_a = np.concatenate([gdst_a, np.zeros((NT, P, pad), np.int32)], 2)
        if Ts < T_sage_all:
            pad = T_sage_all - Ts
            ssrc_a = np.concatenate([ssrc_a, np.zeros((NT, P, pad), np.int32)], 2)
            sslot_a = np.concatenate([sslot_a, np.full((NT, P, pad), -1.0, np.float32)], 2)
        # sage deginv per (bin, slot)
        degs = np.bincount(s_dstperm, minlength=NPpad).astype(np.float32)
        deginv = (1.0 / np.maximum(degs, 1.0)).reshape(NT, P, 1)
        # x shard in permuted order
        x_sh = np.zeros((NPpad, x.shape[1]), np.float32)
        lperm = ggid[k * NPC:(k + 1) * NPC] - k * NPpad
        x_sh[lperm] = x[k * NPC:(k + 1) * NPC]
        # slot row layout [NT, 1, T*P] for the partition-broadcast matmul
        slot_gat_r = np.ascontiguousarray(
            gslot_a.astype(np.float32).transpose(0, 2, 1).reshape(NT, 1, -1))
        slot_sage = np.concatenate([sslot_a.astype(np.float32), deginv], 2)
        cores.append(dict(
            x_sh=x_sh,
            meta_gat=np.ascontiguousarray(gsrc_a.astype(np.int32)),
            slot_gat=np.ascontiguousarray(gslot_a.astype(np.float32)),
            slot_gat_r=slot_gat_r,
            meta_sage=np.ascontiguousarray(ssrc_a.astype(np.int32)),
            slot_sage=np.ascontiguousarray(slot_sage),
        ))

    cfg = dict(n_cores=n_cores, NPC=NPC, NP=NPpad, NT=NT,
               T_gat=T_gat_all, T_sage=T_sage_all, Fin=x.shape[1])
    # host keeps ggid to unpermute outputs
    return cfg, cores, ggid


def fold_weights(W1, a1s, a1d, b1, W2, a2s, a2d, b2, Wl, bl, Wr, M1, mb1, M2, mb2):
    """Host-side weight folding -> replicated device weight arrays."""
    f = lambda a: np.asarray(a, np.float32)
    W1, a1s, a1d, b1 = f(W1), f(a1s), f(a1d), f(b1)
    W2, a2s, a2d, b2 = f(W2), f(a2s), f(a2d), f(b2)
    Wl, bl, Wr, M1, mb1, M2, mb2 = f(Wl), f(bl), f(Wr), f(M1), f(mb1), f(M2), f(mb2)

    def bd(a):  # [HEADS, HID] -> block diag [C, HEADS]
        out = np.zeros((C, HEADS), np.float32)
        for h in range(HEADS):
            out[h * HID:(h + 1) * HID, h] = a[h]
        return out

    w1cat = np.concatenate([W1, W1 @ bd(a1s), W1 @ bd(a1d)], 1)  # [Fin,198]
    w2cat = np.concatenate([W2, W2 @ bd(a2s), W2 @ bd(a2d)], 1)  # [C,198]
    wlmm = Wl @ M1 @ M2                                          # [C,16]
    wrmm = Wr @ M1 @ M2                                          # [C,16]
    cvec = bl @ M1 @ M2 + mb1 @ M2 + mb2                         # [16]
    return dict(
        w1cat=np.ascontiguousarray(w1cat),
        w2cat=np.ascontiguousarray(w2cat),
        wlmm=np.ascontiguousarray(wlmm),
        wrmm=np.ascontiguousarray(wrmm),
        brep1=np.ascontiguousarray(np.tile(b1[None, :], (P, 1))),
        brep2=np.ascontiguousarray(np.tile(b2[None, :], (P, 1))),
        crep=np.ascontiguousarray(np.tile(cvec[None, :], (P, 1))),
    )


def build_program(cfg):
    """Build the Bass/Tile program (SPMD, identical across cores)."""
    import concourse.bass as bass
    import concourse.bacc as bacc
    import concourse.mybir as mybir
    import concourse.tile as tile
    from concourse.masks import make_identity

    n_cores = cfg["n_cores"]
    NP_, NT_, Tg, Ts, Fin = cfg["NP"], cfg["NT"], cfg["T_gat"], cfg["T_sage"], cfg["Fin"]
    NG = n_cores * NP_
    f32 = mybir.dt.float32
    i32 = mybir.dt.int32
    A = mybir.AluOpType
    ACT = mybir.ActivationFunctionType

    nc = bacc.Bacc("TRN2", target_bir_lowering=False, num_devices=n_cores)

    # I/O
    x_in = nc.dram_tensor("x_sh", [NP_, Fin], f32, kind="ExternalInput")
    w1cat = nc.dram_tensor("w1cat", [Fin, C + 2 * HEADS], f32, kind="ExternalInput")
    w2cat = nc.dram_tensor("w2cat", [C, C + 2 * HEADS], f32, kind="ExternalInput")
    wlmm = nc.dram_tensor("wlmm", [C, OUT_C], f32, kind="ExternalInput")
    wrmm = nc.dram_tensor("wrmm", [C, OUT_C], f32, kind="ExternalInput")
    brep1 = nc.dram_tensor("brep1", [P, C], f32, kind="ExternalInput")
    brep2 = nc.dram_tensor("brep2", [P, C], f32, kind="ExternalInput")
    crep = nc.dram_tensor("crep", [P, OUT_C], f32, kind="ExternalInput")
    meta_gat = nc.dram_tensor("meta_gat", [NT_, P, Tg], i32, kind="ExternalInput")
    slot_gat = nc.dram_tensor("slot_gat", [NT_, P, Tg], f32, kind="ExternalInput")
    slot_gat_r = nc.dram_tensor("slot_gat_r", [NT_, 1, Tg * P], f32,
                                kind="ExternalInput")
    meta_sage = nc.dram_tensor("meta_sage", [NT_, P, Ts], i32, kind="ExternalInput")
    slot_sage = nc.dram_tensor("slot_sage", [NT_, P, Ts + 1], f32,
                               kind="ExternalInput")
    out_sh = nc.dram_tensor("out_sh", [NP_, OUT_C], f32, kind="ExternalOutput")

    g1_loc = nc.dram_tensor("g1_loc", [NP_, ROW], f32, kind="Internal")
    ald1 = nc.dram_tensor("ald1", [NP_, HEADS], f32, kind="Internal")
    f2 = nc.dram_tensor("f2", [NP_, C], f32, kind="Internal")
    g2_loc = nc.dram_tensor("g2_loc", [NP_, ROW], f32, kind="Internal")
    ald2 = nc.dram_tensor("ald2", [NP_, HEADS], f32, kind="Internal")
    f3 = nc.dram_tensor("f3", [NP_, C], f32, kind="Internal")
    if n_cores > 1:
        aspace = "Shared" if n_cores > 4 else "Local"
        g1_full = nc.dram_tensor("g1_full", [NG, ROW], f32, kind="Internal",
                                 addr_space=aspace)
        g2_full = nc.dram_tensor("g2_full", [NG, ROW], f32, kind="Internal",
                                 addr_space=aspace)
        f3_full = nc.dram_tensor("f3_full", [NG, C], f32, kind="Internal",
                                 addr_space=aspace)
    else:
        g1_full, g2_full, f3_full = g1_loc, g2_loc, f3

    NC198 = C + 2 * HEADS  # 198

    with tile.TileContext(nc) as tc:
        import contextlib
        ctx = contextlib.ExitStack()
        with ctx:
            cpool = ctx.enter_context(tc.tile_pool(name="const", bufs=1))
            dpool = ctx.enter_context(tc.tile_pool(name="dense", bufs=3))
            epool = ctx.enter_context(tc.tile_pool(name="edge", bufs=2))
            spool = ctx.enter_context(tc.tile_pool(name="spool", bufs=2))
            accps = ctx.enter_context(tc.tile_pool(name="accps", bufs=3, space="PSUM"))
            trps = ctx.enter_context(tc.tile_pool(name="trps", bufs=2, space="PSUM"))
            ops_ps = ctx.enter_context(tc.tile_pool(name="opsps", bufs=2, space="PSUM"))

            # constants
            iota_i = cpool.tile([P, P], i32)
            iota_f = cpool.tile([P, P], f32)
            ident = cpool.tile([P, P], f32)
            nc.gpsimd.iota(iota_i[:], pattern=[[1, P]], base=0, channel_multiplier=0)
            nc.vector.tensor_copy(iota_f[:], iota_i[:])
            make_identity(nc, ident[:])
            # partition-index tile (value = partition id, const along free)
            ipt_i = cpool.tile([P, 1], i32)
            ipt_f = cpool.tile([P, 1], f32)
            nc.gpsimd.iota(ipt_i[:], pattern=[[0, 1]], base=0, channel_multiplier=1)
            nc.vector.tensor_copy(ipt_f[:], ipt_i[:])
            ones_sb = cpool.tile([1, P], f32)
            nc.vector.memset(ones_sb[:], 1.0)

            # resident weights
            w1_sb = cpool.tile([Fin, NC198], f32)
            nc.sync.dma_start(w1_sb[:], w1cat[:, :])
            w2a_sb = cpool.tile([P, NC198], f32)
            w2b_sb = cpool.tile([C - P, NC198], f32)
            nc.sync.dma_start(w2a_sb[:], w2cat[0:P, :])
            nc.sync.dma_start(w2b_sb[:], w2cat[P:C, :])
            wl_a = cpool.tile([P, OUT_C], f32)
            wl_b = cpool.tile([C - P, OUT_C], f32)
            wr_a = cpool.tile([P, OUT_C], f32)
            wr_b = cpool.tile([C - P, OUT_C], f32)
            nc.sync.dma_start(wl_a[:], wlmm[0:P, :])
            nc.sync.dma_start(wl_b[:], wlmm[P:C, :])
            nc.sync.dma_start(wr_a[:], wrmm[0:P, :])
            nc.sync.dma_start(wr_b[:], wrmm[P:C, :])
            b1_sb = cpool.tile([P, C], f32)
            b2_sb = cpool.tile([P, C], f32)
            c_sb = cpool.tile([P, OUT_C], f32)
            nc.sync.dma_start(b1_sb[:], brep1[:, :])
            nc.sync.dma_start(b2_sb[:], brep2[:, :])
            nc.sync.dma_start(c_sb[:], crep[:, :])

            def dense_phase(f_dram, Fin_, wblocks, g_dram, ald_dram, scope):
                # wblocks: list of (sb_tile, k0, kw)
                with nc.named_scope(scope):
                    for c in range(NT_):
                        fsb = dpool.tile([P, Fin_], f32, tag="fsb")
                        nc.sync.dma_start(fsb[:], f_dram[c * P:(c + 1) * P, :])
                        gps = accps.tile([P, NC198], f32, tag="acc")
                        nblk = len(wblocks)
                        for bi, (wt, k0, kw) in enumerate(wblocks):
                            tp = trps.tile([P, P], f32, tag="tp")
                            nc.tensor.transpose(out=tp[:kw, :], in_=fsb[:, k0:k0 + kw],
                                                identity=ident[:])
                            ft = dpool.tile([P, P], f32, tag="ft")
                            nc.vector.tensor_copy(ft[:kw, :], tp[:kw, :])
                            nc.tensor.matmul(out=gps[:], lhsT=ft[:kw, :], rhs=wt[:],
                                             start=(bi == 0), stop=(bi == nblk - 1))
                        gsb = dpool.tile([P, NC198], f32, tag="gsb")
                        nc.vector.tensor_copy(gsb[:], gps[:])
                        nc.sync.dma_start(g_dram[c * P:(c + 1) * P, :], gsb[:, 0:ROW])
                        nc.sync.dma_start(ald_dram[c * P:(c + 1) * P, :],
                                          gsb[:, ROW:NC198])

            def allgather(loc, full, scope):
                with nc.named_scope(scope):
                    nc.gpsimd.collective_compute(
                        "AllGather", A.bypass,
                        replica_groups=[list(range(n_cores))],
                        ins=[loc[:, :]],
                        outs=[full[:, :]],
                    )

            def gat_edge_phase(g_full_d, ald_d, b_sb, f_out, scope):
                with nc.named_scope(scope):
                    for t in range(NT_):
                        mi = epool.tile([P, Tg], i32, tag="mi")
                        nc.sync.dma_start(mi[:], meta_gat[t, :, :])
                        slt = epool.tile([P, Tg], f32, tag="sl")
                        nc.sync.dma_start(slt[:], slot_gat[t, :, :])
                        sl = slt[:, :]
                        slr = epool.tile([1, Tg * P], f32, tag="slr")
                        nc.sync.dma_start(slr[:], slot_gat_r[t, :, :])
                        aldt = epool.tile([P, HEADS], f32, tag="aldt")
                        nc.sync.dma_start(aldt[:], ald_d[t * P:(t + 1) * P, :])
                        G = epool.tile([P, Tg, ROW], f32, tag="G")
                        for j in range(Tg):
                            nc.gpsimd.indirect_dma_start(
                                out=G[:, j, :], out_offset=None, in_=g_full_d[:, :],
                                in_offset=bass.IndirectOffsetOnAxis(
                                    ap=mi[:, j:j + 1], axis=0))
                        # batched one-hot S for all edge tiles: S_all[e,j,d]
                        S_all = spool.tile([P, Tg, P], f32, tag="S")
                        nc.vector.tensor_tensor(
                            out=S_all[:],
                            in0=iota_f[:].unsqueeze(1).broadcast_to([P, Tg, P]),
                            in1=sl.unsqueeze(2).broadcast_to([P, Tg, P]),
                            op=A.is_equal)
                        ps = accps.tile([P, ROW], f32, tag="acc")
                        for j in range(Tg):
                            # al_d routing: S_T[d,e] = (d == slot_e) via
                            # partition-broadcast matmul + is_equal, then
                            # alde[e,:] = S_T.T @ aldt
                            br = trps.tile([P, P], f32, tag="tp")
                            nc.tensor.matmul(out=br[:], lhsT=ones_sb[:],
                                             rhs=slr[:, j * P:(j + 1) * P],
                                             start=True, stop=True)
                            st = epool.tile([P, P], f32, tag="st")
                            nc.vector.tensor_tensor(
                                out=st[:],
                                in0=ipt_f[:].broadcast_to([P, P]),
                                in1=br[:], op=A.is_equal)
                            alde = ops_ps.tile([P, HEADS], f32, tag="small")
                            nc.tensor.matmul(out=alde[:], lhsT=st[:], rhs=aldt[:],
                                             start=True, stop=True)
                            gatt = G[:, j, C:ROW]
                            att = epool.tile([P, HEADS], f32, tag="att")
                            nc.vector.tensor_tensor(out=att[:], in0=gatt,
                                                    in1=alde[:], op=A.add)
                            e2 = epool.tile([P, HEADS], f32, tag="e2")
                            nc.scalar.activation(e2[:], att[:], ACT.Exp, scale=0.2)
                            nc.scalar.activation(gatt, att[:], ACT.Exp)
                            nc.vector.tensor_tensor(out=gatt, in0=gatt, in1=e2[:],
                                                    op=A.max)
                            gh = G[:, j, 0:C].rearrange("p (h c) -> p h c", h=HEADS)
                            gw = G[:, j, C:ROW].to_broadcast([P, HEADS, HID])
                            nc.vector.tensor_tensor(out=gh, in0=gh, in1=gw, op=A.mult)
                            nc.tensor.matmul(out=ps[:], lhsT=S_all[:, j, :],
                                             rhs=G[:, j, :],
                                             start=(j == 0), stop=(j == Tg - 1))
                        zinv = epool.tile([P, HEADS], f32, tag="zinv")
                        nc.vector.reciprocal(zinv[:], ps[:, C:ROW])
                        osb = epool.tile([P, C], f32, tag="osb")
                        nc.vector.tensor_tensor(
                            out=osb[:].rearrange("p (h c) -> p h c", h=HEADS),
                            in0=ps[:, 0:C].rearrange("p (h c) -> p h c", h=HEADS),
                            in1=zinv[:].to_broadcast([P, HEADS, HID]),
                            op=A.mult)
                        nc.vector.tensor_tensor(out=osb[:], in0=osb[:], in1=b_sb[:],
                                                op=A.add)
                        nc.scalar.activation(osb[:], osb[:], ACT.Relu)
                        nc.sync.dma_start(f_out[t * P:(t + 1) * P, :], osb[:])

            def sage_phase(scope):
                with nc.named_scope(scope):
                    for t in range(NT_):
                        mi = epool.tile([P, Ts], i32, tag="mi")
                        nc.sync.dma_start(mi[:], meta_sage[t, :, :])
                        slt = epool.tile([P, Ts + 1], f32, tag="sl")
                        nc.sync.dma_start(slt[:], slot_sage[t, :, :])
                        sl = slt[:, :]
                        G = epool.tile([P, Ts, C], f32, tag="G")
                        for j in range(Ts):
                            nc.gpsimd.indirect_dma_start(
                                out=G[:, j, :], out_offset=None, in_=f3_full[:, :],
                                in_offset=bass.IndirectOffsetOnAxis(
                                    ap=mi[:, j:j + 1], axis=0))
                        S_all = spool.tile([P, Ts, P], f32, tag="S")
                        nc.vector.tensor_tensor(
                            out=S_all[:],
                            in0=iota_f[:].unsqueeze(1).broadcast_to([P, Ts, P]),
                            in1=sl[:, 0:Ts].unsqueeze(2).broadcast_to([P, Ts, P]),
                            op=A.is_equal)
                        ps = accps.tile([P, C], f32, tag="acc")
                        for j in range(Ts):
                            nc.tensor.matmul(out=ps[:], lhsT=S_all[:, j, :],
                                             rhs=G[:, j, :],
                                             start=(j == 0), stop=(j == Ts - 1))
                        asb = epool.tile([P, C], f32, tag="asb")
                        nc.vector.tensor_scalar(out=asb[:], in0=ps[:],
                                                scalar1=sl[:, Ts:Ts + 1], scalar2=None,
                                                op0=A.mult)
                        h2sb = epool.tile([P, C], f32, tag="h2sb")
                        nc.sync.dma_start(h2sb[:], f3[t * P:(t + 1) * P, :])
                        ops = ops_ps.tile([P, OUT_C], f32, tag="small")
                        blocks = [(asb, wl_a, 0, P), (asb, wl_b, P, C - P),
                                  (h2sb, wr_a, 0, P), (h2sb, wr_b, P, C - P)]
                        for bi, (xsb, wt, k0, kw) in enumerate(blocks):
                            tp = trps.tile([P, P], f32, tag="tp")
                            nc.tensor.transpose(out=tp[:kw, :], in_=xsb[:, k0:k0 + kw],
                                                identity=ident[:])
                            xt = epool.tile([P, P], f32, tag="xt")
                            nc.vector.tensor_copy(xt[:kw, :], tp[:kw, :])
                            nc.tensor.matmul(out=ops[:], lhsT=xt[:kw, :], rhs=wt[:],
                                             start=(bi == 0), stop=(bi == 3))
                        fin = epool.tile([P, OUT_C], f32, tag="fin")
                        nc.vector.tensor_tensor(out=fin[:], in0=ops[:], in1=c_sb[:],
                                                op=A.add)
                        nc.scalar.activation(fin[:], fin[:], ACT.Sigmoid)
                        nc.sync.dma_start(out_sh[t * P:(t + 1) * P, :], fin[:])

            # ---- program ----
            dense_phase(x_in, Fin, [(w1_sb, 0, Fin)], g1_loc, ald1, "dense1")
            if n_cores > 1:
                allgather(g1_loc, g1_full, "ag1")
            gat_edge_phase(g1_full, ald1, b1_sb, f2, "edge1")
            dense_phase(f2, C, [(w2a_sb, 0, P), (w2b_sb, P, C - P)], g2_loc, ald2,
                        "dense2")
            if n_cores > 1:
                allgather(g2_loc, g2_full, "ag2")
            gat_edge_phase(g2_full, ald2, b2_sb, f3, "edge2")
            if n_cores > 1:
                allgather(f3, f3_full, "ag3")
            sage_phase("sage")

    nc.compile()
    return nc


LAST_RESULTS = None  # BassKernelResults of the most recent kernel() call


def kernel(**inputs):
    global LAST_RESULTS
    import os
    x = np.asarray(inputs["x"], np.float32)
    edge_index = np.asarray(inputs["edge_index"])
    cfg, cores, ggid = preprocess(x, edge_index, N, NCORES)
    wts = fold_weights(
        inputs["W1"], inputs["a1s"], inputs["a1d"], inputs["b1"],
        inputs["W2"], inputs["a2s"], inputs["a2d"], inputs["b2"],
        inputs["Wl"], inputs["bl"], inputs["Wr"],
        inputs["M1"], inputs["mb1"], inputs["M2"], inputs["mb2"])
    nc = build_program(cfg)
    in_maps = [dict(core, **wts) for core in cores]

    from concourse import bass_utils
    res = bass_utils.run_bass_kernel_spmd(
        nc, in_maps, core_ids=list(range(NCORES)),
        trace=bool(int(os.environ.get("GAT_TRACE", "0"))))
    LAST_RESULTS = res
    NPp = cfg["NP"]
    out = np.zeros((N, OUT_C), np.float32)
    for k in range(NCORES):
        o = res.results[k]["out_sh"]  # [NP, OUT_C]
        lo, hi = k * cfg["NPC"], (k + 1) * cfg["NPC"]
        out[lo:hi] = o[ggid[lo:hi] - k * NPp]
    return out



# revision 20
# speedup vs baseline: 1.8248x; 1.8248x over previous
"""Distributed GATv1 (2x GAT + SAGE + MLP head) for Trainium2, 8 NeuronCores.

v2 design (vs v1 baseline):
- g_full rows are 256 bf16 elems (512B) laid out [h(192) | al_s(3) | al_d(3) |
  pad], written by the dense phase via a zero-padded weight matrix, so one
  table serves the src-row gather and the dst-al_d path.
- Edge gathers use the dma_gather Q7 ucode (one call per (tile, half)) instead
  of per-edge-column indirect DMAs: ~1us Pool time per 1280 rows instead of
  ~1us per 128 rows.  Indices are int16, so nodes are split into two halves
  (gid < NG/2 and >=) and each tile's edge slots are grouped by half.
- Per-edge al_d comes from a one-hot routing matrix st_all[d,(j,e)] built once
  per tile (partition-broadcast matmuls + one is_equal) and Tg tiny matmuls
  against the tile's local al_d rows.
- All per-edge-column element-wise work is batched into a handful of whole-
  tile DVE/ACT ops; everything on the edge path is bf16 (PSUM accums in f32).
"""

import numpy as np

N = 50000
E = 800000
IN_C = 128
HID = 64
HEADS = 3
OUT_C = 16
C = HEADS * HID          # 192
NCORES = 8
P = 128
RW = 256                 # padded row width (bf16) = 512B
ALS0 = C                 # 192: al_s columns
ALD0 = C + HEADS         # 195: al_d columns


def _ceil(a, b):
    return -(-a // b)


def _pack_bins(deg_lo, deg_hi, nbins):
    """Greedy 2D-balanced binning: assign n=nbins*128 nodes to bins of 128
    slots, minimizing the max per-bin edge count for the lo- and hi-half
    source groups separately. Returns (bin_of, slot_of)."""
    n = len(deg_lo)
    assert n == nbins * P
    order = np.argsort(-(deg_lo + deg_hi), kind="stable")
    load_lo = np.zeros(nbins, np.int64)
    load_hi = np.zeros(nbins, np.int64)
    bin_fill = np.zeros(nbins, np.int64)
    bin_of = np.zeros(n, np.int32)
    slot_of = np.zeros(n, np.int32)
    big = np.int64(1 << 60)
    for l in order:
        cand = np.where(bin_fill < P,
                        np.maximum(load_lo + deg_lo[l], load_hi + deg_hi[l]),
                        big)
        b = int(np.argmin(cand))
        bin_of[l] = b
        slot_of[l] = bin_fill[b]
        bin_fill[b] += 1
        load_lo[b] += deg_lo[l]
        load_hi[b] += deg_hi[l]
    assert (bin_fill == P).all()
    return bin_of, slot_of


def _split_tiles(gsrc, dstperm, NT, HALF):
    """Group edges by dst tile, then by src half. Returns per-tile
    (lo_list, hi_list) of (src_gid_in_half, slot) arrays."""
    ebin = (dstperm // P).astype(np.int64)
    eslot = (dstperm % P).astype(np.int64)
    order = np.argsort(ebin, kind="stable")
    counts = np.bincount(ebin, minlength=NT)
    starts = np.zeros(NT + 1, np.int64)
    starts[1:] = np.cumsum(counts)
    out = []
    for t in range(NT):
        sel = order[starts[t]:starts[t + 1]]
        gs, sl = gsrc[sel], eslot[sel]
        m = gs < HALF
        out.append(((gs[m], sl[m]), (gs[~m] - HALF, sl[~m])))
    return out


def _wrap_idx(vals, T):
    """Index list -> dma_gather layout [128, 8*T] i16 (wrapped in 16
    partitions, replicated across the 8 Q7 core groups). Pads with 0."""
    n = T * P
    a = np.zeros(n, np.int16)
    a[:len(vals)] = vals.astype(np.int16)
    blk = a.reshape(n // 16, 16).T          # [16, 8*T]
    out = np.zeros((P, n // 16), np.int16)
    for g in range(8):
        out[g * 16:(g + 1) * 16] = blk
    return out


def _slot_grid(slots, T):
    """Slot list -> [128, T] f32 grid (edge i at [i%128, i//128]); pads -1."""
    n = T * P
    a = np.full(n, -1.0, np.float32)
    a[:len(slots)] = slots.astype(np.float32)
    return a.reshape(T, P).T.copy()         # [128, T]


def preprocess(x, edge_index, n_nodes, n_cores):
    """Host-side index preprocessing. Returns (cfg dict, per-core data, ggid)."""
    import ml_dtypes
    bf = ml_dtypes.bfloat16
    src = np.asarray(edge_index[0], np.int64)
    dst = np.asarray(edge_index[1], np.int64)
    NPC = n_nodes // n_cores
    NPpad = _ceil(NPC, P) * P
    NT = NPpad // P
    NG = n_cores * NPpad
    HALF = NG // 2

    x = np.asarray(x, np.float32)
    owner = dst // NPC
    # in-degree per node split by src half (cores 0..3 are the lo half of the
    # padded-global id space); +1 self-loop counts toward the node's own half
    src_is_lo = (src // NPC) < (n_cores // 2)
    deg_lo = np.bincount(dst[src_is_lo], minlength=n_nodes).astype(np.int64)
    deg_hi = np.bincount(dst[~src_is_lo], minlength=n_nodes).astype(np.int64)
    self_lo = (np.arange(n_nodes) // NPC) < (n_cores // 2)
    deg_lo += self_lo
    deg_hi += ~self_lo

    ggid = np.zeros(n_nodes, np.int64)
    pad_perm = []
    for k in range(n_cores):
        lo, hi = k * NPC, (k + 1) * NPC
        npd = NPpad - NPC
        dlo = np.concatenate([deg_lo[lo:hi], np.ones(npd, np.int64)])
        dhi = np.concatenate([deg_hi[lo:hi], np.zeros(npd, np.int64)])
        b, s = _pack_bins(dlo, dhi, NT)
        ggid[lo:hi] = k * NPpad + b[:NPC].astype(np.int64) * P + s[:NPC]
        pad_perm.append(b[NPC:].astype(np.int64) * P + s[NPC:])

    # per-core per-tile edge groups
    gat_tiles, sage_tiles = [], []
    for k in range(n_cores):
        m = owner == k
        es, ed = src[m], dst[m]
        sl_nodes = np.arange(k * NPC, (k + 1) * NPC, dtype=np.int64)
        ges = np.concatenate([es, sl_nodes])
        ged = np.concatenate([ed, sl_nodes])
        g_src = ggid[ges]
        g_dst = ggid[ged] - k * NPpad
        if len(pad_perm[k]):
            g_src = np.concatenate(
                [g_src, np.full(len(pad_perm[k]), ggid[0], np.int64)])
            g_dst = np.concatenate([g_dst, pad_perm[k]])
        gat_tiles.append(_split_tiles(g_src, g_dst, NT, HALF))
        s_src = ggid[es]
        s_dst = ggid[ed] - k * NPpad
        sage_tiles.append(_split_tiles(s_src, s_dst, NT, HALF))

    def _tmax_per_tile(tiles_all, gi):
        return [max(1, max(_ceil(len(tiles_all[k][t][gi][0]), P)
                           for k in range(n_cores))) for t in range(NT)]

    TGL = _tmax_per_tile(gat_tiles, 0)
    TGH = _tmax_per_tile(gat_tiles, 1)
    TSL = _tmax_per_tile(sage_tiles, 0)
    TSH = _tmax_per_tile(sage_tiles, 1)
    TG = [a + b for a, b in zip(TGL, TGH)]
    TS = [a + b for a, b in zip(TSL, TSH)]
    TGmax, TSmax = max(TG), max(TS)

    cores = []
    for k in range(n_cores):
        # per-tile packed meta: gat [idx_lo|idx_hi|slots], sage [dg|idx|slots]
        metag = np.zeros((NT, P, 9 * TGmax), np.int16)
        slrg = np.zeros((NT, 1, TGmax * P), np.float32)
        metas = np.zeros((NT, P, 2 + 9 * TSmax), np.int16)
        m = owner == k
        s_dst = ggid[dst[m]] - k * NPpad
        degs = np.bincount(s_dst, minlength=NPpad).astype(np.float32)
        deginv = (1.0 / np.maximum(degs, 1.0)).reshape(NT, P)
        for t in range(NT):
            tgl, tgh, tg = TGL[t], TGH[t], TG[t]
            (lg, ls_), (hg, hs) = gat_tiles[k][t]
            metag[t, :, :8 * tgl] = _wrap_idx(lg, tgl)
            metag[t, :, 8 * tgl:8 * tg] = _wrap_idx(hg, tgh)
            slot = np.concatenate(
                [_slot_grid(ls_, tgl), _slot_grid(hs, tgh)], axis=1)
            metag[t, :, 8 * tg:9 * tg] = slot.astype(bf).view(np.int16)
            slrg[t, 0, :tg * P] = slot.T.reshape(-1)
            tsl, tsh, ts_ = TSL[t], TSH[t], TS[t]
            (lg, ls_), (hg, hs) = sage_tiles[k][t]
            metas[t, :, 0:2] = deginv[t].astype(np.float32)[:, None].view(
                np.int16)
            metas[t, :, 2:2 + 8 * tsl] = _wrap_idx(lg, tsl)
            metas[t, :, 2 + 8 * tsl:2 + 8 * ts_] = _wrap_idx(hg, tsh)
            slot = np.concatenate(
                [_slot_grid(ls_, tsl), _slot_grid(hs, tsh)], axis=1)
            metas[t, :, 2 + 8 * ts_:2 + 9 * ts_] = slot.astype(bf).view(np.int16)
        # x shard in permuted order, pretransposed, bf16
        x_sh = np.zeros((NPpad, x.shape[1]), np.float32)
        lperm = ggid[k * NPC:(k + 1) * NPC] - k * NPpad
        x_sh[lperm] = x[k * NPC:(k + 1) * NPC]
        cores.append(dict(
            xT=np.ascontiguousarray(x_sh.T).astype(bf),
            metag=np.ascontiguousarray(metag),
            slrg=np.ascontiguousarray(slrg).astype(bf),
            metas=np.ascontiguousarray(metas),
        ))

    cfg = dict(n_cores=n_cores, NPC=NPC, NP=NPpad, NT=NT, NG=NG, HALF=HALF,
               TGL=TGL, TGH=TGH, TSL=TSL, TSH=TSH,
               TGmax=TGmax, TSmax=TSmax, Fin=x.shape[1])
    return cfg, cores, ggid


def fold_weights(W1, a1s, a1d, b1, W2, a2s, a2d, b2, Wl, bl, Wr, M1, mb1, M2, mb2):
    """Host-side weight folding -> replicated device weight arrays (bf16)."""
    import ml_dtypes
    bf = ml_dtypes.bfloat16
    f = lambda a: np.asarray(a, np.float32)
    W1, a1s, a1d, b1 = f(W1), f(a1s), f(a1d), f(b1)
    W2, a2s, a2d, b2 = f(W2), f(a2s), f(a2d), f(b2)
    Wl, bl, Wr, M1, mb1, M2, mb2 = f(Wl), f(bl), f(Wr), f(M1), f(mb1), f(M2), f(mb2)

    def bd(a):  # [HEADS, HID] -> block diag [C, HEADS]
        out = np.zeros((C, HEADS), np.float32)
        for h in range(HEADS):
            out[h * HID:(h + 1) * HID, h] = a[h]
        return out

    def pad256(w):  # [K, 198] -> [K, 256]
        out = np.zeros((w.shape[0], RW), np.float32)
        out[:, :w.shape[1]] = w
        return out

    # feature permutation: h-major (h*64+c) -> channel-major (c*3+h), so that
    # per-head broadcasts on device have stride-1 innermost (DVE 2x mode)
    perm = np.array([(k % HEADS) * HID + k // HEADS for k in range(C)])

    w1cat = pad256(np.concatenate([W1[:, perm], W1 @ bd(a1s), W1 @ bd(a1d)], 1))
    w2cat = pad256(np.concatenate([W2[:, perm], W2 @ bd(a2s), W2 @ bd(a2d)], 1))
    w2cat[:C] = w2cat[perm]          # rows follow f2's (c,h) order
    wlmm = (Wl @ M1 @ M2)[perm]
    wrmm = (Wr @ M1 @ M2)[perm]
    cvec = bl @ M1 @ M2 + mb1 @ M2 + mb2
    return dict(
        w1cat=np.ascontiguousarray(w1cat).astype(bf),
        w2cat=np.ascontiguousarray(w2cat).astype(bf),
        wlmm=np.ascontiguousarray(wlmm).astype(bf),
        wrmm=np.ascontiguousarray(wrmm).astype(bf),
        brep1=np.ascontiguousarray(np.tile(b1[None, perm], (P, 1))).astype(bf),
        brep2=np.ascontiguousarray(np.tile(b2[None, perm], (P, 1))).astype(bf),
        crep=np.ascontiguousarray(np.tile(cvec[None, :], (P, 1))),
    )


def build_program(cfg):
    """Build the Bass/Tile program (SPMD, identical across cores)."""
    import concourse.bass as bass
    import concourse.bacc as bacc
    import concourse.mybir as mybir
    import concourse.tile as tile
    from concourse.masks import make_identity
    from concourse import library_config

    n_cores = cfg["n_cores"]
    NP_, NT_ = cfg["NP"], cfg["NT"]
    NG, HALF = cfg["NG"], cfg["HALF"]
    TGL, TGH, TSL, TSH = cfg["TGL"], cfg["TGH"], cfg["TSL"], cfg["TSH"]
    TG = [a + b for a, b in zip(TGL, TGH)]
    TS = [a + b for a, b in zip(TSL, TSH)]
    TGmax, TSmax = cfg["TGmax"], cfg["TSmax"]
    Fin = cfg["Fin"]
    f32 = mybir.dt.float32
    bf16 = mybir.dt.bfloat16
    i16 = mybir.dt.int16
    i32 = mybir.dt.int32
    A = mybir.AluOpType
    ACT = mybir.ActivationFunctionType

    nc = bacc.Bacc("TRN2", target_bir_lowering=False, num_devices=n_cores)

    # I/O
    xT_in = nc.dram_tensor("xT", [Fin, NP_], bf16, kind="ExternalInput")
    w1cat = nc.dram_tensor("w1cat", [Fin, RW], bf16, kind="ExternalInput")
    w2cat = nc.dram_tensor("w2cat", [C, RW], bf16, kind="ExternalInput")
    wlmm = nc.dram_tensor("wlmm", [C, OUT_C], bf16, kind="ExternalInput")
    wrmm = nc.dram_tensor("wrmm", [C, OUT_C], bf16, kind="ExternalInput")
    brep1 = nc.dram_tensor("brep1", [P, C], bf16, kind="ExternalInput")
    brep2 = nc.dram_tensor("brep2", [P, C], bf16, kind="ExternalInput")
    crep = nc.dram_tensor("crep", [P, OUT_C], f32, kind="ExternalInput")
    metag = nc.dram_tensor("metag", [NT_, P, 9 * TGmax], i16,
                           kind="ExternalInput")
    slrg = nc.dram_tensor("slrg", [NT_, 1, TGmax * P], bf16,
                          kind="ExternalInput")
    metas = nc.dram_tensor("metas", [NT_, P, 2 + 9 * TSmax], i16,
                           kind="ExternalInput")
    out_sh = nc.dram_tensor("out_sh", [NP_, OUT_C], f32, kind="ExternalOutput")

    g1_loc = nc.dram_tensor("g1_loc", [NP_, RW], bf16, kind="Internal")
    f2 = nc.dram_tensor("f2", [NP_, C], bf16, kind="Internal")
    g2_loc = nc.dram_tensor("g2_loc", [NP_, RW], bf16, kind="Internal")
    f3_loc = nc.dram_tensor("f3_loc", [NP_, RW], bf16, kind="Internal")
    if n_cores > 1:
        aspace = "Shared" if n_cores > 4 else "Local"
        g1_full = nc.dram_tensor("g1_full", [NG, RW], bf16, kind="Internal",
                                 addr_space=aspace)
        g2_full = nc.dram_tensor("g2_full", [NG, RW], bf16, kind="Internal",
                                 addr_space=aspace)
        f3_full = nc.dram_tensor("f3_full", [NG, RW], bf16, kind="Internal",
                                 addr_space=aspace)
    else:
        g1_full, g2_full, f3_full = g1_loc, g2_loc, f3_loc


    with tile.TileContext(nc) as tc:
        import contextlib
        ctx = contextlib.ExitStack()
        with ctx:
            cpool = ctx.enter_context(tc.tile_pool(name="const", bufs=1))
            dpool = ctx.enter_context(tc.tile_pool(name="dense", bufs=4))
            epool = ctx.enter_context(tc.tile_pool(name="edge", bufs=4))
            spool = ctx.enter_context(tc.tile_pool(name="spool", bufs=4))
            accps = ctx.enter_context(tc.tile_pool(name="accps", bufs=2, space="PSUM"))
            brps = ctx.enter_context(tc.tile_pool(name="brps", bufs=2, space="PSUM"))
            tpps = ctx.enter_context(tc.tile_pool(name="tpps", bufs=2, space="PSUM"))
            smps = ctx.enter_context(tc.tile_pool(name="smps", bufs=2, space="PSUM"))

            nc.gpsimd.load_library(library_config.mlp)

            # constants
            iota_i = cpool.tile([P, P], i32)
            iota_b = cpool.tile([P, P], bf16)
            nc.gpsimd.iota(iota_i[:], pattern=[[1, P]], base=0, channel_multiplier=0)
            nc.vector.tensor_copy(iota_b[:], iota_i[:])
            ident_f = cpool.tile([P, P], f32)
            ident_b = cpool.tile([P, P], bf16)
            make_identity(nc, ident_f[:])
            nc.vector.tensor_copy(ident_b[:], ident_f[:])
            ipt_i = cpool.tile([P, 512], i32)
            ipt_b = cpool.tile([P, 512], bf16)
            nc.gpsimd.iota(ipt_i[:], pattern=[[0, 512]], base=0,
                           channel_multiplier=1)
            nc.vector.tensor_copy(ipt_b[:], ipt_i[:])
            ones_b = cpool.tile([1, P], bf16)
            nc.vector.memset(ones_b[:], 1.0)

            # resident weights
            w1_sb = cpool.tile([Fin, RW], bf16)
            nc.sync.dma_start(w1_sb[:], w1cat[:, :])
            w2a_sb = cpool.tile([P, RW], bf16)
            w2b_sb = cpool.tile([C - P, RW], bf16)
            nc.sync.dma_start(w2a_sb[:], w2cat[0:P, :])
            nc.sync.dma_start(w2b_sb[:], w2cat[P:C, :])
            wl_a = cpool.tile([P, OUT_C], bf16)
            wl_b = cpool.tile([C - P, OUT_C], bf16)
            wr_a = cpool.tile([P, OUT_C], bf16)
            wr_b = cpool.tile([C - P, OUT_C], bf16)
            nc.sync.dma_start(wl_a[:], wlmm[0:P, :])
            nc.sync.dma_start(wl_b[:], wlmm[P:C, :])
            nc.sync.dma_start(wr_a[:], wrmm[0:P, :])
            nc.sync.dma_start(wr_b[:], wrmm[P:C, :])
            b1_sb = cpool.tile([P, C], bf16)
            b2_sb = cpool.tile([P, C], bf16)
            c_sb = cpool.tile([P, OUT_C], f32)
            nc.sync.dma_start(b1_sb[:], brep1[:, :])
            nc.sync.dma_start(b2_sb[:], brep2[:, :])
            nc.sync.dma_start(c_sb[:], crep[:, :])

            def dense1(scope):
                XB = 4
                with nc.named_scope(scope):
                    for t0 in range(0, NT_, XB):
                        nb = min(XB, NT_ - t0)
                        xt = dpool.tile([P, XB * P], bf16, tag="xt")
                        nc.sync.dma_start(xt[:, 0:nb * P],
                                          xT_in[:, t0 * P:(t0 + nb) * P])
                        for i in range(nb):
                            t = t0 + i
                            gps = accps.tile([P, RW], f32, tag="acc")
                            nc.tensor.matmul(out=gps[:],
                                             lhsT=xt[:, i * P:(i + 1) * P],
                                             rhs=w1_sb[:], start=True, stop=True)
                            gsb = dpool.tile([P, RW], bf16, tag="gsb")
                            nc.scalar.copy(gsb[:], gps[:])
                            nc.sync.dma_start(g1_loc[t * P:(t + 1) * P, :], gsb[:])

            def dense2(scope):
                with nc.named_scope(scope):
                    for t in range(NT_):
                        fsb = dpool.tile([P, C], bf16, tag="fsb")
                        nc.sync.dma_start(fsb[:], f2[t * P:(t + 1) * P, :])
                        gps = accps.tile([P, RW], f32, tag="acc")
                        for bi, (wt, k0, kw) in enumerate(
                                [(w2a_sb, 0, P), (w2b_sb, P, C - P)]):
                            tp = tpps.tile([P, P], bf16, tag="tp")
                            nc.tensor.transpose(out=tp[:kw, :],
                                                in_=fsb[:, k0:k0 + kw],
                                                identity=ident_b[:])
                            ft = dpool.tile([P, P], bf16, tag="ft")
                            nc.scalar.copy(ft[:kw, :], tp[:kw, :])
                            nc.tensor.matmul(out=gps[:], lhsT=ft[:kw, :], rhs=wt[:],
                                             start=(bi == 0), stop=(bi == 1))
                        gsb = dpool.tile([P, RW], bf16, tag="gsb")
                        nc.scalar.copy(gsb[:], gps[:])
                        nc.sync.dma_start(g2_loc[t * P:(t + 1) * P, :], gsb[:])

            def allgather(loc, full, scope):
                with nc.named_scope(scope):
                    nc.gpsimd.collective_compute(
                        "AllGather", A.bypass,
                        replica_groups=[list(range(n_cores))],
                        ins=[loc[:, :]],
                        outs=[full[:, :]],
                    )

            def gat_edge(g_full_d, g_loc_d, b_sb, f_out, fo_width, scope):
                with nc.named_scope(scope):
                    for t in range(NT_):
                        tgl, tgh, tg = TGL[t], TGH[t], TG[t]
                        gw_ = 9 * tg
                        meta = epool.tile([P, gw_ + (gw_ % 2)], i16, tag="mi")
                        nc.sync.dma_start(meta[:, 0:gw_], metag[t, :, 0:gw_])
                        mi = meta[:, 0:8 * tg]
                        sl = meta[:, 8 * tg:9 * tg].bitcast(bf16)
                        slr = epool.tile([1, tg * P], bf16, tag="slr")
                        nc.sync.dma_start(slr[:], slrg[t, :, 0:tg * P])
                        aldt = epool.tile([P, HEADS], bf16, tag="aldt")
                        nc.sync.dma_start(
                            aldt[:], g_loc_d[t * P:(t + 1) * P, ALD0:ALD0 + HEADS])
                        G = epool.tile([P, tg, RW], bf16, tag="G")
                        for h0, hw_, tbl in (
                                (0, tgl, g_full_d[0:HALF, :]),
                                (tgl, tgh, g_full_d[HALF:NG, :])):
                            for c0 in range(0, hw_, 8):
                                ck = min(8, hw_ - c0)
                                j0 = h0 + c0
                                nc.gpsimd.dma_gather(
                                    G[:, j0:j0 + ck, :], tbl,
                                    mi[:, 8 * j0:8 * (j0 + ck)],
                                    ck * P, ck * P, RW)
                        # st_all[d, (j,e)] = (d == slot[e,j])
                        st_all = spool.tile([P, tg * P], bf16, tag="st")
                        brs = epool.tile([P, tg * P], bf16, tag="brs")
                        for c0 in range(0, tg * P, 512):
                            cw = min(512, tg * P - c0)
                            br = brps.tile([P, 512], f32, tag="br")
                            nc.tensor.matmul(out=br[:, 0:cw], lhsT=ones_b[:],
                                             rhs=slr[:, c0:c0 + cw],
                                             start=True, stop=True)
                            nc.scalar.copy(brs[:, c0:c0 + cw], br[:, 0:cw])
                        nc.vector.tensor_tensor(
                            out=st_all[:].rearrange("p (t e) -> p t e", e=P),
                            in0=ipt_b[:, 0:P].unsqueeze(1).broadcast_to(
                                [P, tg, P]),
                            in1=brs[:].rearrange("p (t e) -> p t e", e=P),
                            op=A.is_equal)
                        # alde[(e), (j,h)] via tg tiny matmuls
                        alde = smps.tile([P, tg * HEADS], f32, tag="sm")
                        for j in range(tg):
                            nc.tensor.matmul(
                                out=alde[:, j * HEADS:(j + 1) * HEADS],
                                lhsT=st_all[:, j * P:(j + 1) * P], rhs=aldt[:],
                                start=True, stop=True)
                        # attention weights w = exp(leaky_relu(al_s + al_d))
                        att = epool.tile([P, tg, HEADS], f32, tag="att")
                        nc.vector.tensor_tensor(
                            out=att[:], in0=G[:, :, ALS0:ALS0 + HEADS],
                            in1=alde[:].rearrange("p (t h) -> p t h", h=HEADS),
                            op=A.add)
                        e2 = epool.tile([P, tg, HEADS], f32, tag="e2")
                        nc.scalar.activation(e2[:], att[:], ACT.Exp, scale=0.2)
                        gw = G[:, :, ALS0:ALS0 + HEADS]
                        nc.scalar.activation(gw, att[:], ACT.Exp)
                        nc.vector.tensor_tensor(out=gw, in0=gw, in1=e2[:], op=A.max)
                        # scale messages: G[:, :, 0:C] *= w (per head)
                        gh = G[:, :, 0:C].rearrange("p t (c h) -> p t c h", h=HEADS)
                        gwb = G[:, :, ALS0:ALS0 + HEADS].unsqueeze(2).broadcast_to(
                            [P, tg, HID, HEADS])
                        nc.vector.tensor_tensor(out=gh, in0=gh, in1=gwb, op=A.mult)
                        # one-hot S and aggregation
                        S_all = spool.tile([P, tg, P], bf16, tag="S")
                        nc.vector.tensor_tensor(
                            out=S_all[:],
                            in0=iota_b[:].unsqueeze(1).broadcast_to([P, tg, P]),
                            in1=sl.unsqueeze(2).broadcast_to([P, tg, P]),
                            op=A.is_equal)
                        ps = accps.tile([P, ALD0], f32, tag="acc")
                        for j in range(tg):
                            nc.tensor.matmul(out=ps[:], lhsT=S_all[:, j, :],
                                             rhs=G[:, j, 0:ALD0],
                                             start=(j == 0), stop=(j == tg - 1))
                        zinv = epool.tile([P, HEADS], f32, tag="zinv")
                        nc.vector.reciprocal(zinv[:], ps[:, ALS0:ALD0])
                        osb = epool.tile([P, C], bf16, tag="osb")
                        nc.vector.tensor_tensor(
                            out=osb[:].rearrange("p (c h) -> p c h", h=HEADS),
                            in0=ps[:, 0:ALS0].rearrange("p (c h) -> p c h", h=HEADS),
                            in1=zinv[:].unsqueeze(1).broadcast_to([P, HID, HEADS]),
                            op=A.mult)
                        nc.vector.tensor_tensor(out=osb[:], in0=osb[:], in1=b_sb[:],
                                                op=A.add)
                        nc.scalar.activation(osb[:], osb[:], ACT.Relu)
                        nc.sync.dma_start(f_out[t * P:(t + 1) * P, 0:C], osb[:])

            def sage(scope):
                with nc.named_scope(scope):
                    for t in range(NT_):
                        tsl, tsh, ts_ = TSL[t], TSH[t], TS[t]
                        mw = 2 + 9 * ts_
                        meta = epool.tile([P, mw + (mw % 2)], i16, tag="mi")
                        nc.sync.dma_start(meta[:, 0:mw], metas[t, :, 0:mw])
                        dg = meta[:, 0:2].bitcast(f32)
                        mi = meta[:, 2:2 + 8 * ts_]
                        sl = meta[:, 2 + 8 * ts_:2 + 9 * ts_].bitcast(bf16)
                        G = epool.tile([P, ts_, RW], bf16, tag="G")
                        for h0, hw_, tbl in (
                                (0, tsl, f3_full[0:HALF, :]),
                                (tsl, tsh, f3_full[HALF:NG, :])):
                            for c0 in range(0, hw_, 8):
                                ck = min(8, hw_ - c0)
                                j0 = h0 + c0
                                nc.gpsimd.dma_gather(
                                    G[:, j0:j0 + ck, :], tbl,
                                    mi[:, 8 * j0:8 * (j0 + ck)],
                                    ck * P, ck * P, RW)
                        S_all = spool.tile([P, ts_, P], bf16, tag="S")
                        nc.vector.tensor_tensor(
                            out=S_all[:],
                            in0=iota_b[:].unsqueeze(1).broadcast_to([P, ts_, P]),
                            in1=sl.unsqueeze(2).broadcast_to([P, ts_, P]),
                            op=A.is_equal)
                        ps = accps.tile([P, C], f32, tag="acc")
                        for j in range(ts_):
                            nc.tensor.matmul(out=ps[:], lhsT=S_all[:, j, :],
                                             rhs=G[:, j, 0:C],
                                             start=(j == 0), stop=(j == ts_ - 1))
                        asb = epool.tile([P, C], bf16, tag="asb")
                        nc.vector.tensor_scalar(out=asb[:], in0=ps[:],
                                                scalar1=dg, scalar2=None,
                                                op0=A.mult)
                        h2sb = epool.tile([P, C], bf16, tag="h2sb")
                        nc.sync.dma_start(h2sb[:], f3_loc[t * P:(t + 1) * P, 0:C])
                        ops = smps.tile([P, OUT_C], f32, tag="sm")
                        blocks = [(asb, wl_a, 0, P), (asb, wl_b, P, C - P),
                                  (h2sb, wr_a, 0, P), (h2sb, wr_b, P, C - P)]
                        for bi, (xsb, wt, k0, kw) in enumerate(blocks):
                            tp = tpps.tile([P, P], bf16, tag="tp")
                            nc.tensor.transpose(out=tp[:kw, :], in_=xsb[:, k0:k0 + kw],
                                                identity=ident_b[:])
                            xt = epool.tile([P, P], bf16, tag="xt")
                            nc.scalar.copy(xt[:kw, :], tp[:kw, :])
                            nc.tensor.matmul(out=ops[:], lhsT=xt[:kw, :], rhs=wt[:],
                                             start=(bi == 0), stop=(bi == 3))
                        fin = epool.tile([P, OUT_C], f32, tag="fin")
                        nc.vector.tensor_tensor(out=fin[:], in0=ops[:], in1=c_sb[:],
                                                op=A.add)
                        nc.scalar.activation(fin[:], fin[:], ACT.Sigmoid)
                        nc.sync.dma_start(out_sh[t * P:(t + 1) * P, :], fin[:])

            # ---- program ----
            dense1("dense1")
            if n_cores > 1:
                allgather(g1_loc, g1_full, "ag1")
            gat_edge(g1_full, g1_loc, b1_sb, f2, C, "edge1")
            dense2("dense2")
            if n_cores > 1:
                allgather(g2_loc, g2_full, "ag2")
            gat_edge(g2_full, g2_loc, b2_sb, f3_loc, RW, "edge2")
            if n_cores > 1:
                allgather(f3_loc, f3_full, "ag3")
            sage("sage")

    nc.compile()
    return nc


LAST_RESULTS = None  # BassKernelResults of the most recent kernel() call


def kernel(**inputs):
    global LAST_RESULTS
    import os
    x = np.asarray(inputs["x"], np.float32)
    edge_index = np.asarray(inputs["edge_index"])
    cfg, cores, ggid = preprocess(x, edge_index, N, NCORES)
    wts = fold_weights(
        inputs["W1"], inputs["a1s"], inputs["a1d"], inputs["b1"],
        inputs["W2"], inputs["a2s"], inputs["a2d"], inputs["b2"],
        inputs["Wl"], inputs["bl"], inputs["Wr"],
        inputs["M1"], inputs["mb1"], inputs["M2"], inputs["mb2"])
    nc = build_program(cfg)
    in_maps = [dict(core, **wts) for core in cores]

    from concourse import bass_utils
    res = bass_utils.run_bass_kernel_spmd(
        nc, in_maps, core_ids=list(range(NCORES)),
        trace=bool(int(os.environ.get("GAT_TRACE", "0"))))
    LAST_RESULTS = res
    NPp = cfg["NP"]
    out = np.zeros((N, OUT_C), np.float32)
    for k in range(NCORES):
        o = res.results[k]["out_sh"]  # [NP, OUT_C]
        lo, hi = k * cfg["NPC"], (k + 1) * cfg["NPC"]
        out[lo:hi] = o[ggid[lo:hi] - k * NPp]
    return out
